# revision 12
# baseline (speedup 1.0000x reference)
# Trainium2 Bass kernel for nn_ColorConsistencyMetric.
#
# Reference computation (B=32, C=3, H=W=1024, GRID=4):
#   region_means[b,c,gi,gj] = mean of the 256x256 block (gi,gj) of images[b,c]
#   color_std[b] = mean_c std(region_means[b,c,:], ddof=1)
#   out = mean_b 1/(1+color_std[b])
#
# Strategy: pure data parallel over the batch dim across 8 NeuronCores
# (4 images per core). Each core streams its 48 MiB shard through SBUF
# (12 channel-images, one 4 MiB DMA each as a [128, 8192] tile: partition
# p holds image rows 8p..8p+7), computes per-(row-in-partition, col-block)
# sums with a single segmented VectorE reduce, reduces across partitions
# with a block-diagonal ones matmul on TensorE (partitions 32m..32m+31 all
# belong to block-row m), and a tiny second reduce yields the 16 block
# sums per channel-image. The 8x[4,48] outputs (one f32 per block per
# channel-image) are combined on the host: block mean -> std -> 1/(1+std)
# -> mean over batch. The kernel is HBM-bandwidth bound (~48 MiB/core).

import numpy as np

_B, _C, _H, _W = 32, 3, 1024, 1024
_GRID = 4
_NCORES = 8
_BPC = _B // _NCORES            # images per core
_NIMG = _BPC * _C               # channel-images per core
_RPP = _H // 128                # image rows per SBUF partition
_FD = _RPP * _W                 # free dim of one channel-image tile
_BLK = (_H // _GRID) * (_W // _GRID)  # pixels per block

_cache = {}


def _build_bass(repeats=1):
    """repeats>1 re-runs the whole per-core workload inside one program;
    used by test.py to difference out the host->device dispatch overhead
    when timing. kernel() always uses repeats=1."""
    import concourse.bass as bass
    import concourse.bacc as bacc
    import concourse.tile as tile
    from concourse import mybir

    nc = bacc.Bacc(
        "TRN2",
        target_bir_lowering=False,
        debug=False,
        num_devices=_NCORES,
    )
    imgs = nc.dram_tensor(
        "images", [_NIMG, 128, _FD], mybir.dt.float32, kind="ExternalInput"
    ).ap()
    out = nc.dram_tensor(
        "blocksums",
        [_GRID, _NIMG * _GRID * repeats],
        mybir.dt.float32,
        kind="ExternalOutput",
    ).ap()

    with tile.TileContext(nc) as tc:
        with (
            tc.tile_pool(name="big", bufs=4) as big,
            tc.tile_pool(name="rows", bufs=4) as rows,
            tc.tile_pool(name="psum", bufs=4, space="PSUM") as psum_pool,
            tc.tile_pool(name="const", bufs=1) as const_pool,
            tc.tile_pool(name="outp", bufs=1) as outp,
        ):
            # Block-diagonal ones: lhsT[p, m] = 1 iff p // 32 == m, so the
            # matmul sums partitions within each block-row group.
            lhsT = const_pool.tile([128, _GRID], mybir.dt.float32)
            nc.vector.memset(lhsT, 0.0)
            for m in range(_GRID):
                nc.vector.memset(lhsT[m * 32 : (m + 1) * 32, m : m + 1], 1.0)

            out_tile = outp.tile(
                [_GRID, _NIMG * _GRID * repeats], mybir.dt.float32
            )

            for k in range(_NIMG * repeats):
                i = k % _NIMG
                t = big.tile([128, _FD], mybir.dt.float32)
                nc.sync.dma_start(out=t, in_=imgs[i])
                # Per-partition segmented sums: row r (of the 8 in this
                # partition), col-block j -> rs[p, r*GRID + j].
                rs = rows.tile([128, _RPP * _GRID], mybir.dt.float32)
                nc.vector.reduce_sum(
                    out=rs,
                    in_=t.rearrange("p (r j c) -> p r j c", r=_RPP, j=_GRID),
                    axis=mybir.AxisListType.X,
                )
                # Sum the 128 partitions within each block-row group.
                ps = psum_pool.tile([_GRID, _RPP * _GRID], mybir.dt.float32)
                nc.tensor.matmul(ps, lhsT, rs, start=True, stop=True)
                # Sum over the 8 rows-per-partition -> 16 block sums.
                nc.vector.reduce_sum(
                    out=out_tile[:, k * _GRID : (k + 1) * _GRID],
                    in_=ps.rearrange("m (r j) -> m j r", r=_RPP),
                    axis=mybir.AxisListType.X,
                )
            # Single copy so the output DMA depends on one instruction
            # instead of 12 (DMA sync-wait slots are limited).
            out_tile2 = outp.tile(
                [_GRID, _NIMG * _GRID * repeats], mybir.dt.float32
            )
            nc.vector.tensor_copy(out_tile2, out_tile)
            nc.sync.dma_start(out=out, in_=out_tile2)
    nc.compile()
    return nc


def _get_nc(repeats=1):
    key = ("nc", repeats)
    if key not in _cache:
        _cache[key] = _build_bass(repeats)
    return _cache[key]


def _run_on_device(images_np, trace=False, **spmd_kwargs):
    from concourse.bass_utils import run_bass_kernel_spmd

    nc = _get_nc()
    in_maps = []
    for c in range(_NCORES):
        shard = np.ascontiguousarray(
            images_np[c * _BPC : (c + 1) * _BPC], dtype=np.float32
        ).reshape(_NIMG, 128, _FD)
        in_maps.append({"images": shard})
    res = run_bass_kernel_spmd(
        nc, in_maps, core_ids=list(range(_NCORES)), trace=trace, **spmd_kwargs
    )
    return res


def _finish_host(block_sum_list):
    """block_sum_list: per-core [GRID, NIMG*GRID] arrays of block sums."""
    cons = []
    for o in block_sum_list:
        # o[gi, i*GRID + gj] with i = local_b * C + c
        M = np.asarray(o, dtype=np.float64).reshape(_GRID, _NIMG, _GRID)
        sums = M.transpose(1, 0, 2)                      # (i, gi, gj)
        means = (sums / _BLK).reshape(_BPC, _C, _GRID * _GRID)
        mu = means.mean(axis=-1, keepdims=True)
        var = ((means - mu) ** 2).sum(axis=-1) / (_GRID * _GRID - 1)
        std = np.sqrt(var)                               # (b, c)
        color_std = std.mean(axis=1)                     # (b,)
        cons.append(1.0 / (1.0 + color_std))
    return np.array(np.concatenate(cons).mean(), dtype=np.float32)


def kernel(images):
    images_np = np.asarray(images)
    res = _run_on_device(images_np, trace=False)
    outs = [r["blocksums"] for r in res.results]
    return _finish_host(outs)
